# revision 10
# baseline (speedup 1.0000x reference)
"""ConvGRU Trainium2 kernel.

video [B=2, T=16, C=128, H=64, W=64] f32; 1x1-conv GRU over T.
Sharding: data-parallel over (B x H/4) -> 8 cores, each core owns
P = 16*64 = 1024 pixels for all T; weights replicated.

Layout per core: channels on partitions (128), pixels on the free dim.
Two pixel groups (G=2, PG=512) give two independent recurrence chains
that interleave across engines.

PSUM (8 banks):
  r_psum [128,1024] halves = groups   (2 banks, single-buffered)
  z_psum [128,1024] halves = groups   (2 banks, single-buffered)
  c_psum [128,1024] halves = groups   (2 banks x 2 ping-pong)

Per steady step t:
  PE : WRH.h_g (x2), WZH.h_all, WHX'.x(t+1), WHH.rh_g (x2),
       WRX'.x(t+1), WZX'.x(t+1)  [+ optional heater re-runs of opens]
  ACT: sig_r_g0, sig_r_g1 [512], zbar_all [1024] (scale=-1, bias=-bz),
       tanh_g0, tanh_g1 [512]
  DVE: rh_g = r*h (x2), u_all = zbar*h, z_all = 1-zbar,
       v_g = z*c (x2), h'_g = u+v (x2)
  DMA: x(t+2) in on SP queue; h'_g out on gpsimd queue.

t=0 is specialized (h=0): no h-side matmuls, no r sigmoid, no rh/u/add;
z = sigmoid(+pre_z+bz) directly and h(0) = z*c.

A warmup block (dense matmuls + dummy sigmoid) ramps the PE clock and
loads the ACT table while the first x DMA is in flight.

Numerics: fp16 matmul inputs/gates/state, fp32 PSUM accum + fp32 bias.
"""

import os
import sys

import numpy as np

B, T, C, H, W = 2, 16, 128, 64, 64
NCORES = 8
HQ = H // 4          # 16 rows of H per core (4 H-slices x 2 batches = 8 cores)
P = HQ * W           # 1024 pixels per core
G = 2                # pixel groups per step (independent recurrence chains)
PG = P // G          # 512 pixels per group

N_HEAT = 2           # extra re-runs of x-side opens to hold the PE p-state

_PROG = None


def _ensure_paths():
    for p in ("/opt/trn_rl_repo",):
        if p not in sys.path and os.path.isdir(p):
            sys.path.append(p)


def _build():
    _ensure_paths()
    import concourse.bacc as bacc
    import concourse.tile as tile
    from concourse import mybir

    f32 = mybir.dt.float32
    f16 = mybir.dt.float16
    AF = mybir.ActivationFunctionType

    nc = bacc.Bacc(
        "TRN2", target_bir_lowering=False, debug=False, num_devices=NCORES
    )
    x_dram = nc.dram_tensor("x_seq", [T, C, P], f16, kind="ExternalInput")
    w_dram = nc.dram_tensor("wmats", [C, 6 * C], f16, kind="ExternalInput")
    b_dram = nc.dram_tensor("biases", [C, 4], f32, kind="ExternalInput")
    o_dram = nc.dram_tensor("out_seq", [T, C, P], f16, kind="ExternalOutput")

    x_ap = x_dram.ap()
    w_ap = w_dram.ap()
    b_ap = b_dram.ap()
    o_ap = o_dram.ap()

    WZX, WZH, WRX, WRH, WHX, WHH = range(6)
    # bias columns: [br, -bz, bh, +bz]
    BR, NBZ, BH, PBZ = range(4)

    def gslice(ap_, g):
        return ap_[:, g * PG : (g + 1) * PG]

    with tile.TileContext(nc) as tc:
        with (
            tc.tile_pool(name="consts", bufs=1) as consts,
            tc.tile_pool(name="xin", bufs=4) as xpool,
            tc.tile_pool(name="state", bufs=2) as spool,
            tc.tile_pool(name="work", bufs=2) as wk,
            tc.tile_pool(name="ps", bufs=1, space="PSUM") as ps,
        ):
            wt = consts.tile([C, 6 * C], f16)
            nc.sync.dma_start(wt[:], w_ap[:])
            bt = consts.tile([C, 4], f32)
            nc.gpsimd.dma_start(bt[:], b_ap[:])

            def wslice(i):
                return wt[:, i * C : (i + 1) * C]

            # persistent PSUM accumulators
            r_ps = ps.tile([C, P], f32, tag="r_ps", bufs=1)
            z_ps = ps.tile([C, P], f32, tag="z_ps", bufs=1)

            def c_tile():
                return ps.tile([C, P], f32, tag="c_ps", bufs=2, name="c_ps")

            # fp16 state [h_g0 | h_g1]
            def h_tile():
                return spool.tile([C, P], f16, tag="h16", name="h16")

            # -- warmup: ramp the PE clock gate + preload the ACT table
            #    while the first x DMA is in flight --
            c_ps = c_tile()
            for i in range(6):
                nc.tensor.matmul(
                    c_ps[:, :PG], wslice(i % 6), wt[:, :PG],
                    start=True, stop=True,
                )
            wtmp = wk.tile([C, P], f16, tag="r16")
            nc.scalar.activation(
                gslice(wtmp, 0), c_ps[:, :PG], AF.Sigmoid, bias=bt[:, BR : BR + 1]
            )

            def load_x(t):
                xt = xpool.tile([C, P], f16, tag="x")
                nc.sync.dma_start(xt[:], x_ap[t])
                return xt

            # NOTE: a matmul output cannot cross a PSUM bank boundary, so
            # every matmul is a [C, PG] chunk (one bank).
            def open_c(xt, c_ps_new):
                for g in range(G):
                    nc.tensor.matmul(
                        gslice(c_ps_new, g), wslice(WHX), gslice(xt, g),
                        start=True, stop=False,
                    )

            def open_rz(xt, heat=0):
                """x-side z/r contributions: start fresh accum groups."""
                for _ in range(heat):
                    nc.tensor.matmul(gslice(r_ps, 0), wslice(WRX), gslice(xt, 0),
                                     start=True, stop=False, skip_group_check=True)
                for g in range(G):
                    nc.tensor.matmul(
                        gslice(r_ps, g), wslice(WRX), gslice(xt, g),
                        start=True, stop=False,
                        skip_group_check=(heat > 0 and g == 0),
                    )
                for g in range(G):
                    nc.tensor.matmul(
                        gslice(z_ps, g), wslice(WZX), gslice(xt, g),
                        start=True, stop=False,
                    )

            def open_all(xt, c_ps_new, heat=0):
                open_rz(xt, heat)
                open_c(xt, c_ps_new)

            # ---------------- t = 0 (h = 0) ----------------
            x_t = load_x(0)
            x_n = load_x(1)
            # only z and c pre-acts are needed at t=0 (h=0 kills the rest)
            for g in range(G):
                nc.tensor.matmul(gslice(z_ps, g), wslice(WZX), gslice(x_t, g),
                                 start=True, stop=True)
                nc.tensor.matmul(gslice(c_ps, g), wslice(WHX), gslice(x_t, g),
                                 start=True, stop=True)
            # z = sigmoid(pre_z + bz); c = tanh(pre_c + bh); h0 = z*c
            z16 = wk.tile([C, P], f16, tag="z16")
            nc.scalar.activation(z16[:], z_ps[:], AF.Sigmoid, bias=bt[:, PBZ : PBZ + 1])
            c16 = wk.tile([C, P], f16, tag="c16")
            for g in range(G):
                nc.scalar.activation(
                    gslice(c16, g), gslice(c_ps, g), AF.Tanh, bias=bt[:, BH : BH + 1]
                )
            h16 = h_tile()
            for g in range(G):
                nc.vector.tensor_mul(gslice(h16, g), gslice(z16, g), gslice(c16, g))
                nc.gpsimd.dma_start(o_ap[0, :, g * PG : (g + 1) * PG], gslice(h16, g))

            # open step 1 accumulators
            c_pn = c_tile()
            open_all(x_n, c_pn)
            x_t, c_ps = x_n, c_pn

            # ---------------- steady steps ----------------
            for t in range(1, T):
                go = (0, 1) if t % 2 else (1, 0)
                x_n = load_x(t + 1) if t + 1 < T else None

                # --- PE: close r then z accumulations (chain head) ---
                for g in go:
                    nc.tensor.matmul(
                        gslice(r_ps, g), wslice(WRH), gslice(h16, g),
                        start=False, stop=True,
                    )
                for g in go:
                    nc.tensor.matmul(
                        gslice(z_ps, g), wslice(WZH), gslice(h16, g),
                        start=False, stop=True,
                    )

                # --- ACT: r sigmoids first (they gate rh -> c matmul) ---
                r16 = wk.tile([C, P], f16, tag="r16")
                for g in go:
                    nc.scalar.activation(
                        gslice(r16, g), gslice(r_ps, g), AF.Sigmoid,
                        bias=bt[:, BR : BR + 1],
                    )
                # zbar = 1 - z = sigmoid(-(pre_z + bz)) -- off the r chain
                zb16 = wk.tile([C, P], f16, tag="zb16")
                nc.scalar.activation(
                    zb16[:], z_ps[:], AF.Sigmoid,
                    bias=bt[:, NBZ : NBZ + 1], scale=-1.0,
                )

                # --- DVE: rh, then off-chain u/z while tanh runs ---
                rh16 = wk.tile([C, P], f16, tag="rh16")
                for g in go:
                    nc.vector.tensor_mul(gslice(rh16, g), gslice(r16, g), gslice(h16, g))

                # --- PE: open next step's c (fills the rh-wait gap), then
                #     close this step's c accumulation, then open next r/z ---
                c_pn = c_tile() if x_n is not None else None
                if x_n is not None:
                    open_c(x_n, c_pn)
                for g in go:
                    nc.tensor.matmul(
                        gslice(c_ps, g), wslice(WHH), gslice(rh16, g),
                        start=False, stop=True,
                    )
                if x_n is not None:
                    open_rz(x_n, heat=N_HEAT)

                # --- ACT: tanh per group ---
                c16 = wk.tile([C, P], f16, tag="c16")
                for g in go:
                    nc.scalar.activation(
                        gslice(c16, g), gslice(c_ps, g), AF.Tanh,
                        bias=bt[:, BH : BH + 1],
                    )

                # --- DVE: u = zbar*h, z = 1-zbar (off-chain) ---
                u16 = wk.tile([C, P], f16, tag="u16")
                nc.vector.tensor_mul(u16[:], zb16[:], h16[:])
                z16 = wk.tile([C, P], f16, tag="z16")
                nc.vector.tensor_scalar(
                    z16[:], zb16[:], -1.0, 1.0,
                    mybir.AluOpType.mult, mybir.AluOpType.add,
                )

                # --- DVE: blend tail v = z*c, h' = u + v ---
                h_new = h_tile()
                v16 = wk.tile([C, P], f16, tag="v16")
                for g in go:
                    nc.vector.tensor_mul(gslice(v16, g), gslice(z16, g), gslice(c16, g))
                    nc.vector.tensor_add(gslice(h_new, g), gslice(u16, g), gslice(v16, g))
                    nc.gpsimd.dma_start(
                        o_ap[t, :, g * PG : (g + 1) * PG], gslice(h_new, g)
                    )

                h16 = h_new
                if x_n is not None:
                    x_t, c_ps = x_n, c_pn

    nc.compile()
    return nc


def _get_prog():
    global _PROG
    if _PROG is None:
        _PROG = _build()
    return _PROG


def _make_in_maps(video, Wz, bz, Wr, br, Wh, bh):
    w6 = np.concatenate(
        [
            Wz[:, :C].T, Wz[:, C:].T,
            Wr[:, :C].T, Wr[:, C:].T,
            Wh[:, :C].T, Wh[:, C:].T,
        ],
        axis=1,
    ).astype(np.float16)
    b4 = np.stack([br, -bz, bh, bz], axis=1).astype(np.float32)
    in_maps = []
    for core in range(NCORES):
        b_, q = divmod(core, 4)
        xs = np.ascontiguousarray(
            video[b_, :, :, q * HQ : (q + 1) * HQ, :]
        ).reshape(T, C, P).astype(np.float16)
        in_maps.append({"x_seq": xs, "wmats": w6, "biases": b4})
    return in_maps


def kernel(video, Wz, bz, Wr, br, Wh, bh):
    _ensure_paths()
    from concourse.bass_utils import run_bass_kernel_spmd

    video = np.asarray(video, dtype=np.float32)
    nc = _get_prog()
    in_maps = _make_in_maps(video, Wz, bz, Wr, br, Wh, bh)
    res = run_bass_kernel_spmd(nc, in_maps, list(range(NCORES)))

    out = np.empty((B, T, C, H, W), np.float32)
    for core in range(NCORES):
        b_, q = divmod(core, 4)
        out[b_, :, :, q * HQ : (q + 1) * HQ, :] = np.asarray(
            res.results[core]["out_seq"]
        ).astype(np.float32).reshape(T, C, HQ, W)
    return out


# revision 11
# speedup vs baseline: 1.1834x; 1.1834x over previous
"""ConvGRU Trainium2 kernel.

video [B=2, T=16, C=128, H=64, W=64] f32; 1x1-conv GRU over T.
Sharding: data-parallel over (B x H/4) -> 8 cores, each core owns
P = 16*64 = 1024 pixels for all T; weights replicated.

Layout per core: channels on partitions (128), pixels on the free dim.
Two pixel groups (G=2, PG=512) form two FULLY DECOUPLED recurrence
chains (no shared instructions) so they can phase-offset and keep every
engine busy; group blocks are issued alternately each step.

PSUM (8 banks):
  r_ps [128,1024] halves = groups   (2 banks, single-buffered)
  z_ps [128,1024] halves = groups   (2 banks, single-buffered)
  c_ps [128,1024] halves = groups   (2 banks x 2 ping-pong)

Per step t, per group g (issue order; engines in brackets):
  open_c'_g(t+1)                [PE]   (c ping-pong bank, x prefetched)
  WRH_g, WZH_g                  [PE]   close r/z pre-acts with h(t-1)
  sig_r_g; zbar_g = sig(-pre-bz)[ACT]
  rh_g = r*h                    [DVE]
  u_g  = zbar*h                 [DVE]
  z_g  = 1-zbar                 [Pool] (frees DVE)
  open_r'_g(t+1)                [PE]   (after sig_r_g drains the bank)
  WHH_g                         [PE]   close c pre-act with rh
  open_z'_g(t+1)                [PE]
  tanh_g                        [ACT]
  v_g = z*c; h'_g = u+v         [DVE]
  out dma                       [Pool queue]

t=0 is specialized (h=0): only z/c pre-acts, z = sigmoid(+pre+bz),
h(0) = z*c.  x DMAs run two steps ahead on the SP queue.

Numerics: fp16 matmul inputs/gates/state, fp32 PSUM accum + fp32 bias.
"""

import os
import sys

import numpy as np

B, T, C, H, W = 2, 16, 128, 64, 64
NCORES = 8
HQ = H // 4          # 16 rows of H per core (4 H-slices x 2 batches = 8 cores)
P = HQ * W           # 1024 pixels per core
G = 2                # pixel groups per step (independent recurrence chains)
PG = P // G          # 512 pixels per group

_PROG = None


def _ensure_paths():
    for p in ("/opt/trn_rl_repo",):
        if p not in sys.path and os.path.isdir(p):
            sys.path.append(p)


def _build():
    _ensure_paths()
    import concourse.bacc as bacc
    import concourse.tile as tile
    from concourse import mybir

    f32 = mybir.dt.float32
    f16 = mybir.dt.float16
    AF = mybir.ActivationFunctionType

    nc = bacc.Bacc(
        "TRN2", target_bir_lowering=False, debug=False, num_devices=NCORES
    )
    x_dram = nc.dram_tensor("x_seq", [T, C, P], f16, kind="ExternalInput")
    w_dram = nc.dram_tensor("wmats", [C, 6 * C], f16, kind="ExternalInput")
    b_dram = nc.dram_tensor("biases", [C, 4], f32, kind="ExternalInput")
    o_dram = nc.dram_tensor("out_seq", [T, C, P], f16, kind="ExternalOutput")

    x_ap = x_dram.ap()
    w_ap = w_dram.ap()
    b_ap = b_dram.ap()
    o_ap = o_dram.ap()

    WZX, WZH, WRX, WRH, WHX, WHH = range(6)
    # bias columns: [br, -bz, bh, +bz]
    BR, NBZ, BH, PBZ = range(4)

    def gslice(ap_, g):
        return ap_[:, g * PG : (g + 1) * PG]

    with tile.TileContext(nc) as tc:
        with (
            tc.tile_pool(name="consts", bufs=1) as consts,
            tc.tile_pool(name="xin", bufs=4) as xpool,
            tc.tile_pool(name="state", bufs=2) as spool,
            tc.tile_pool(name="work", bufs=2) as wk,
            tc.tile_pool(name="ps", bufs=1, space="PSUM") as ps,
        ):
            wt = consts.tile([C, 6 * C], f16)
            nc.sync.dma_start(wt[:], w_ap[:])
            bt = consts.tile([C, 4], f32)
            nc.gpsimd.dma_start(bt[:], b_ap[:])

            def wslice(i):
                return wt[:, i * C : (i + 1) * C]

            # persistent PSUM accumulators
            r_ps = ps.tile([C, P], f32, tag="r_ps", bufs=1)
            z_ps = ps.tile([C, P], f32, tag="z_ps", bufs=1)

            def c_tile():
                return ps.tile([C, P], f32, tag="c_ps", bufs=2, name="c_ps")

            def h_tile():
                return spool.tile([C, P], f16, tag="h16", name="h16")

            def wtile(tag):
                return wk.tile([C, P], f16, tag=tag, name=tag)

            # -- warmup: ramp the PE clock gate + preload the ACT table
            #    while the first x DMA is in flight --
            c_ps = c_tile()
            for i in range(6):
                nc.tensor.matmul(
                    c_ps[:, :PG], wslice(i % 6), wt[:, :PG],
                    start=True, stop=True,
                )
            wtmp = wtile("r16")
            nc.scalar.activation(
                gslice(wtmp, 0), c_ps[:, :PG], AF.Sigmoid, bias=bt[:, BR : BR + 1]
            )

            def load_x(t):
                xt = xpool.tile([C, P], f16, tag="x", name="x")
                nc.sync.dma_start(xt[:], x_ap[t])
                return xt

            # ---------------- t = 0 (h = 0) ----------------
            xs = {0: load_x(0), 1: load_x(1)}
            x_t = xs[0]
            # only z and c pre-acts are needed at t=0 (h=0 kills the rest)
            for g in range(G):
                nc.tensor.matmul(gslice(z_ps, g), wslice(WZX), gslice(x_t, g),
                                 start=True, stop=True)
                nc.tensor.matmul(gslice(c_ps, g), wslice(WHX), gslice(x_t, g),
                                 start=True, stop=True)
            xs[2] = load_x(2)
            z16 = wtile("z16")
            c16 = wtile("c16")
            h16 = h_tile()
            for g in range(G):
                nc.scalar.activation(gslice(z16, g), gslice(z_ps, g), AF.Sigmoid,
                                     bias=bt[:, PBZ : PBZ + 1])
                nc.scalar.activation(gslice(c16, g), gslice(c_ps, g), AF.Tanh,
                                     bias=bt[:, BH : BH + 1])
                nc.vector.tensor_mul(gslice(h16, g), gslice(z16, g), gslice(c16, g))
                nc.gpsimd.dma_start(o_ap[0, :, g * PG : (g + 1) * PG], gslice(h16, g))

            # open step 1 accumulators (x side)
            c_pn = c_tile()
            x_n = xs[1]
            for g in range(G):
                nc.tensor.matmul(gslice(c_pn, g), wslice(WHX), gslice(x_n, g),
                                 start=True, stop=False)
                nc.tensor.matmul(gslice(r_ps, g), wslice(WRX), gslice(x_n, g),
                                 start=True, stop=False)
                nc.tensor.matmul(gslice(z_ps, g), wslice(WZX), gslice(x_n, g),
                                 start=True, stop=False)
            c_ps = c_pn

            # ---------------- steady steps ----------------
            for t in range(1, T):
                go = (0, 1) if t % 2 else (1, 0)
                opens = t + 1 < T
                x_n = xs[t + 1] if opens else None
                if t + 2 < T:
                    xs[t + 2] = load_x(t + 2)
                c_pn = c_tile() if opens else None

                r16 = wtile("r16")
                zb16 = wtile("zb16")
                z16 = wtile("z16")
                u16 = wtile("u16")
                rh16 = wtile("rh16")
                c16 = wtile("c16")
                v16 = wtile("v16")
                h_new = h_tile()

                for g in go:
                    # PE: next step's c opener (no deps beyond x prefetch),
                    # then close this step's r/z pre-acts
                    if opens:
                        nc.tensor.matmul(
                            gslice(c_pn, g), wslice(WHX), gslice(x_n, g),
                            start=True, stop=False,
                        )
                    nc.tensor.matmul(
                        gslice(r_ps, g), wslice(WRH), gslice(h16, g),
                        start=False, stop=True,
                    )
                    nc.tensor.matmul(
                        gslice(z_ps, g), wslice(WZH), gslice(h16, g),
                        start=False, stop=True,
                    )

                    # ACT: r gate (chain head), then zbar (off-chain)
                    nc.scalar.activation(
                        gslice(r16, g), gslice(r_ps, g), AF.Sigmoid,
                        bias=bt[:, BR : BR + 1],
                    )
                    nc.scalar.activation(
                        gslice(zb16, g), gslice(z_ps, g), AF.Sigmoid,
                        bias=bt[:, NBZ : NBZ + 1], scale=-1.0,
                    )

                    # DVE: rh (chain), u (off-chain); Pool: z = 1-zbar
                    nc.vector.tensor_mul(
                        gslice(rh16, g), gslice(r16, g), gslice(h16, g)
                    )
                    nc.vector.tensor_mul(
                        gslice(u16, g), gslice(zb16, g), gslice(h16, g)
                    )
                    nc.gpsimd.tensor_scalar(
                        gslice(z16, g), gslice(zb16, g), -1.0, 1.0,
                        mybir.AluOpType.mult, mybir.AluOpType.add,
                    )

                    # PE: next r opener (bank freed by sig_r), close c, next z
                    if opens:
                        nc.tensor.matmul(
                            gslice(r_ps, g), wslice(WRX), gslice(x_n, g),
                            start=True, stop=False,
                        )
                    nc.tensor.matmul(
                        gslice(c_ps, g), wslice(WHH), gslice(rh16, g),
                        start=False, stop=True,
                    )
                    if opens:
                        nc.tensor.matmul(
                            gslice(z_ps, g), wslice(WZX), gslice(x_n, g),
                            start=True, stop=False,
                        )

                    # ACT: tanh; DVE: blend tail; out DMA
                    nc.scalar.activation(
                        gslice(c16, g), gslice(c_ps, g), AF.Tanh,
                        bias=bt[:, BH : BH + 1],
                    )
                    nc.vector.tensor_mul(
                        gslice(v16, g), gslice(z16, g), gslice(c16, g)
                    )
                    nc.vector.tensor_add(
                        gslice(h_new, g), gslice(u16, g), gslice(v16, g)
                    )
                    nc.gpsimd.dma_start(
                        o_ap[t, :, g * PG : (g + 1) * PG], gslice(h_new, g)
                    )

                h16 = h_new
                if opens:
                    c_ps = c_pn

    nc.compile()
    return nc


def _get_prog():
    global _PROG
    if _PROG is None:
        _PROG = _build()
    return _PROG


def _make_in_maps(video, Wz, bz, Wr, br, Wh, bh):
    w6 = np.concatenate(
        [
            Wz[:, :C].T, Wz[:, C:].T,
            Wr[:, :C].T, Wr[:, C:].T,
            Wh[:, :C].T, Wh[:, C:].T,
        ],
        axis=1,
    ).astype(np.float16)
    b4 = np.stack([br, -bz, bh, bz], axis=1).astype(np.float32)
    in_maps = []
    for core in range(NCORES):
        b_, q = divmod(core, 4)
        xs = np.ascontiguousarray(
            video[b_, :, :, q * HQ : (q + 1) * HQ, :]
        ).reshape(T, C, P).astype(np.float16)
        in_maps.append({"x_seq": xs, "wmats": w6, "biases": b4})
    return in_maps


def kernel(video, Wz, bz, Wr, br, Wh, bh):
    _ensure_paths()
    from concourse.bass_utils import run_bass_kernel_spmd

    video = np.asarray(video, dtype=np.float32)
    nc = _get_prog()
    in_maps = _make_in_maps(video, Wz, bz, Wr, br, Wh, bh)
    res = run_bass_kernel_spmd(nc, in_maps, list(range(NCORES)))

    out = np.empty((B, T, C, H, W), np.float32)
    for core in range(NCORES):
        b_, q = divmod(core, 4)
        out[b_, :, :, q * HQ : (q + 1) * HQ, :] = np.asarray(
            res.results[core]["out_seq"]
        ).astype(np.float32).reshape(T, C, HQ, W)
    return out


# revision 12
# speedup vs baseline: 1.4064x; 1.1885x over previous
"""ConvGRU Trainium2 kernel.

video [B=2, T=16, C=128, H=64, W=64] f32; 1x1-conv GRU over T.
Sharding: data-parallel over (B x H/4) -> 8 cores, each core owns
P = 16*64 = 1024 pixels for all T; weights replicated.

Layout per core: channels on partitions (128), pixels on the free dim.
Two pixel groups (G=2, PG=512) run as two independent recurrence
chains, SOFTWARE-PIPELINED half an iteration apart: each iteration
emits group A's full step t plus group B's deferred back-half of step
t-1 and B's front-half of step t.  This keeps every engine queue fed
with ready work (in-order engines never block one chain on the other).

PSUM (8 banks):
  r_ps [128,1024] halves = groups   (2 banks, single-buffered)
  z_ps [128,1024] halves = groups   (2 banks, single-buffered)
  c_ps [128,1024] halves = groups   (2 banks x 2 ping-pong)

Stage contents per group g, step t:
  front: WRH_g,WZH_g [PE] -> sig_r_g, zbar_g [ACT] -> rh_g,u_g [DVE],
         z_g=1-zbar [Pool]
  back : WHH_g [PE] -> tanh_g [ACT] -> v_g=z*c, h'_g=u+v [DVE],
         out-DMA [Pool queue]
  opens: x-side matmuls for t+1 (r/z after the gate reads free the
         banks; c into the ping-pong buffer)

t=0 is specialized (h=0).  x DMAs run two steps ahead on the SP queue.
Numerics: fp16 matmul inputs/gates/state, fp32 PSUM accum + fp32 bias.
"""

import os
import sys

import numpy as np

B, T, C, H, W = 2, 16, 128, 64, 64
NCORES = 8
HQ = H // 4          # 16 rows of H per core (4 H-slices x 2 batches = 8 cores)
P = HQ * W           # 1024 pixels per core
G = 2
PG = P // G          # 512 pixels per group

_PROG = None


def _ensure_paths():
    for p in ("/opt/trn_rl_repo",):
        if p not in sys.path and os.path.isdir(p):
            sys.path.append(p)


def _build():
    _ensure_paths()
    import concourse.bacc as bacc
    import concourse.tile as tile
    from concourse import mybir

    f32 = mybir.dt.float32
    f16 = mybir.dt.float16
    AF = mybir.ActivationFunctionType

    nc = bacc.Bacc(
        "TRN2", target_bir_lowering=False, debug=False, num_devices=NCORES
    )
    x_dram = nc.dram_tensor("x_seq", [T, C, P], f16, kind="ExternalInput")
    w_dram = nc.dram_tensor("wmats", [C, 6 * C], f16, kind="ExternalInput")
    b_dram = nc.dram_tensor("biases", [C, 4], f32, kind="ExternalInput")
    o_dram = nc.dram_tensor("out_seq", [T, C, P], f16, kind="ExternalOutput")

    x_ap = x_dram.ap()
    w_ap = w_dram.ap()
    b_ap = b_dram.ap()
    o_ap = o_dram.ap()

    WZX, WZH, WRX, WRH, WHX, WHH = range(6)
    # bias columns: [br, -bz, bh, +bz]
    BR, NBZ, BH, PBZ = range(4)

    def gs(ap_, g):
        return ap_[:, g * PG : (g + 1) * PG]

    with tile.TileContext(nc) as tc:
        with (
            tc.tile_pool(name="consts", bufs=1) as consts,
            tc.tile_pool(name="xin", bufs=4) as xpool,
            tc.tile_pool(name="state", bufs=2) as spool,
            tc.tile_pool(name="work", bufs=2) as wk,
            tc.tile_pool(name="ps", bufs=1, space="PSUM") as ps,
        ):
            wt = consts.tile([C, 6 * C], f16)
            nc.sync.dma_start(wt[:], w_ap[:])
            bt = consts.tile([C, 4], f32)
            nc.gpsimd.dma_start(bt[:], b_ap[:])

            def wslice(i):
                return wt[:, i * C : (i + 1) * C]

            r_ps = ps.tile([C, P], f32, tag="r_ps", bufs=1)
            z_ps = ps.tile([C, P], f32, tag="z_ps", bufs=1)

            def c_tile():
                return ps.tile([C, P], f32, tag="c_ps", bufs=2, name="c_ps")

            def gtile(tag):
                return wk.tile([C, PG], f16, tag=tag, name=tag)

            def htile(g):
                return spool.tile([C, PG], f16, tag=f"h16{g}", name=f"h16{g}")

            # -- warmup: ramp the PE clock gate + preload the ACT table --
            c_cur = c_tile()
            for i in range(6):
                nc.tensor.matmul(
                    c_cur[:, :PG], wslice(i % 6), wt[:, :PG],
                    start=True, stop=True,
                )
            wtmp = gtile("r16_0")
            nc.scalar.activation(
                wtmp[:], c_cur[:, :PG], AF.Sigmoid, bias=bt[:, BR : BR + 1]
            )

            def load_x(t):
                xt = xpool.tile([C, P], f16, tag="x", name="x")
                nc.sync.dma_start(xt[:], x_ap[t])
                return xt

            # ---- pipeline stage emitters ----
            def front(g, t, xnext, c_next, h_prev):
                """r/z pre-act close + gates + rh/u/z for (g, t).
                Returns ctx needed by back()."""
                nc.tensor.matmul(gs(r_ps, g), wslice(WRH), h_prev[:],
                                 start=False, stop=True)
                nc.tensor.matmul(gs(z_ps, g), wslice(WZH), h_prev[:],
                                 start=False, stop=True)
                r16 = gtile(f"r16_{g}")
                zb16 = gtile(f"zb16_{g}")
                nc.scalar.activation(r16[:], gs(r_ps, g), AF.Sigmoid,
                                     bias=bt[:, BR : BR + 1])
                nc.scalar.activation(zb16[:], gs(z_ps, g), AF.Sigmoid,
                                     bias=bt[:, NBZ : NBZ + 1], scale=-1.0)
                rh16 = gtile(f"rh16_{g}")
                u16 = gtile(f"u16_{g}")
                z16 = gtile(f"z16_{g}")
                nc.vector.tensor_mul(rh16[:], r16[:], h_prev[:])
                nc.vector.tensor_mul(u16[:], zb16[:], h_prev[:])
                nc.gpsimd.tensor_scalar(z16[:], zb16[:], -1.0, 1.0,
                                        mybir.AluOpType.mult,
                                        mybir.AluOpType.add)
                # next step's r/z x-side opens go later (after the gate
                # reads); they are emitted by opens_rz().
                return {"rh": rh16, "u": u16, "z": z16, "t": t}

            def back_pe(g, ctx, c_ps_t):
                nc.tensor.matmul(gs(c_ps_t, g), wslice(WHH), ctx["rh"][:],
                                 start=False, stop=True)

            def back_rest(g, ctx, c_ps_t):
                t = ctx["t"]
                c16 = gtile(f"c16_{g}")
                nc.scalar.activation(c16[:], gs(c_ps_t, g), AF.Tanh,
                                     bias=bt[:, BH : BH + 1])
                v16 = gtile(f"v16_{g}")
                h_new = htile(g)
                nc.vector.tensor_mul(v16[:], ctx["z"][:], c16[:])
                nc.vector.tensor_add(h_new[:], ctx["u"][:], v16[:])
                nc.gpsimd.dma_start(o_ap[t, :, g * PG : (g + 1) * PG],
                                    h_new[:])
                return h_new

            def open_c(g, xt, c_ps_new):
                nc.tensor.matmul(gs(c_ps_new, g), wslice(WHX), gs(xt, g),
                                 start=True, stop=False)

            def open_rz(g, xt):
                nc.tensor.matmul(gs(r_ps, g), wslice(WRX), gs(xt, g),
                                 start=True, stop=False)
                nc.tensor.matmul(gs(z_ps, g), wslice(WZX), gs(xt, g),
                                 start=True, stop=False)

            # ---------------- t = 0 (h = 0) ----------------
            xs = {0: load_x(0), 1: load_x(1)}
            x0 = xs[0]
            for g in range(G):
                nc.tensor.matmul(gs(z_ps, g), wslice(WZX), gs(x0, g),
                                 start=True, stop=True)
                nc.tensor.matmul(gs(c_cur, g), wslice(WHX), gs(x0, g),
                                 start=True, stop=True)
            xs[2] = load_x(2)
            h_a = None
            h_b = None
            for g in range(G):
                z16 = gtile(f"z16_{g}")
                c16 = gtile(f"c16_{g}")
                nc.scalar.activation(z16[:], gs(z_ps, g), AF.Sigmoid,
                                     bias=bt[:, PBZ : PBZ + 1])
                nc.scalar.activation(c16[:], gs(c_cur, g), AF.Tanh,
                                     bias=bt[:, BH : BH + 1])
                hg = htile(g)
                nc.vector.tensor_mul(hg[:], z16[:], c16[:])
                nc.gpsimd.dma_start(o_ap[0, :, g * PG : (g + 1) * PG], hg[:])
                if g == 0:
                    h_a = hg
                else:
                    h_b = hg

            # opens for step 1 (both groups; banks are free)
            c_cur = c_tile()
            for g in range(G):
                open_c(g, xs[1], c_cur)
                open_rz(g, xs[1])

            # ---------------- pipeline prologue (iter 1) ----------------
            # A full step 1; B front of step 1; opens for 2.
            actx = front(0, 1, xs[2], None, h_a)
            back_pe(0, actx, c_cur)
            back_rest_h = back_rest(0, actx, c_cur)
            h_a = back_rest_h
            bctx = front(1, 1, xs[2], None, h_b)
            c_next = c_tile()
            xs[3] = load_x(3)
            open_c(0, xs[2], c_next)
            open_c(1, xs[2], c_next)
            open_rz(0, xs[2])
            open_rz(1, xs[2])
            c_prev, c_cur = c_cur, c_next

            # ---------------- steady iterations tau = 2..T-1 ----------------
            for t in range(2, T):
                opens = t + 1 < T
                x_n = xs[t + 1] if opens else None
                if t + 2 < T:
                    xs[t + 2] = load_x(t + 2)

                # A front (step t)
                new_actx = front(0, t, x_n, None, h_a)
                # B back (step t-1) -- deps all ready
                back_pe(1, bctx, c_prev)
                # A back (step t)
                back_pe(0, new_actx, c_cur)
                h_b = back_rest(1, bctx, c_prev)
                h_a = back_rest(0, new_actx, c_cur)
                # B front (step t)
                new_bctx = front(1, t, x_n, None, h_b)

                # opens for t+1
                if opens:
                    c_next = c_tile()
                    open_c(0, x_n, c_next)
                    open_rz(0, x_n)
                    open_c(1, x_n, c_next)
                    open_rz(1, x_n)
                    c_prev, c_cur = c_cur, c_next
                else:
                    c_prev = c_cur

                actx, bctx = new_actx, new_bctx

            # ---------------- epilogue: B back of step T-1 ----------------
            back_pe(1, bctx, c_prev)
            back_rest(1, bctx, c_prev)

    nc.compile()
    return nc


def _get_prog():
    global _PROG
    if _PROG is None:
        _PROG = _build()
    return _PROG


def _make_in_maps(video, Wz, bz, Wr, br, Wh, bh):
    w6 = np.concatenate(
        [
            Wz[:, :C].T, Wz[:, C:].T,
            Wr[:, :C].T, Wr[:, C:].T,
            Wh[:, :C].T, Wh[:, C:].T,
        ],
        axis=1,
    ).astype(np.float16)
    b4 = np.stack([br, -bz, bh, bz], axis=1).astype(np.float32)
    in_maps = []
    for core in range(NCORES):
        b_, q = divmod(core, 4)
        xsl = np.ascontiguousarray(
            video[b_, :, :, q * HQ : (q + 1) * HQ, :]
        ).reshape(T, C, P).astype(np.float16)
        in_maps.append({"x_seq": xsl, "wmats": w6, "biases": b4})
    return in_maps


def kernel(video, Wz, bz, Wr, br, Wh, bh):
    _ensure_paths()
    from concourse.bass_utils import run_bass_kernel_spmd

    video = np.asarray(video, dtype=np.float32)
    nc = _get_prog()
    in_maps = _make_in_maps(video, Wz, bz, Wr, br, Wh, bh)
    res = run_bass_kernel_spmd(nc, in_maps, list(range(NCORES)))

    out = np.empty((B, T, C, H, W), np.float32)
    for core in range(NCORES):
        b_, q = divmod(core, 4)
        out[b_, :, :, q * HQ : (q + 1) * HQ, :] = np.asarray(
            res.results[core]["out_seq"]
        ).astype(np.float32).reshape(T, C, HQ, W)
    return out


# revision 16
# speedup vs baseline: 1.4977x; 1.0649x over previous
"""ConvGRU Trainium2 kernel.

video [B=2, T=16, C=128, H=64, W=64] f32; 1x1-conv GRU over T.
Sharding: data-parallel over (B x H/4) -> 8 cores, each core owns
P = 16*64 = 1024 pixels for all T; weights replicated.

Layout per core: channels on partitions (128), pixels on the free dim.
Two pixel groups (G=2, PG=512) run as two independent recurrence
chains, SOFTWARE-PIPELINED half an iteration apart: each iteration
emits group A's full step t plus group B's deferred back-half of step
t-1 and B's front-half of step t.  This keeps every engine queue fed
with ready work (in-order engines never block one chain on the other).

PSUM (8 banks):
  r_ps [128,1024] halves = groups   (2 banks, single-buffered)
  z_ps [128,1024] halves = groups   (2 banks, single-buffered)
  c_ps [128,1024] halves = groups   (2 banks x 2 ping-pong)

Stage contents per group g, step t:
  front: WRH_g,WZH_g [PE] -> sig_r_g, zbar_g [ACT] -> rh_g,u_g [DVE],
         z_g=1-zbar [Pool]
  back : WHH_g [PE] -> tanh_g [ACT] -> v_g=z*c, h'_g=u+v [DVE],
         out-DMA [Pool queue]
  opens: x-side matmuls for t+1 (r/z after the gate reads free the
         banks; c into the ping-pong buffer)

t=0 is specialized (h=0).  x DMAs run two steps ahead on the SP queue.
Numerics: fp16 matmul inputs/gates/state, fp32 PSUM accum + fp32 bias.
"""

import os
import sys

import numpy as np

B, T, C, H, W = 2, 16, 128, 64, 64
NCORES = 8
HQ = H // 4          # 16 rows of H per core (4 H-slices x 2 batches = 8 cores)
P = HQ * W           # 1024 pixels per core
G = 2
PG = P // G          # 512 pixels per group

_PROG = None


def _ensure_paths():
    for p in ("/opt/trn_rl_repo",):
        if p not in sys.path and os.path.isdir(p):
            sys.path.append(p)


def _build():
    _ensure_paths()
    import concourse.bacc as bacc
    import concourse.tile as tile
    from concourse import mybir

    f32 = mybir.dt.float32
    f16 = mybir.dt.float16
    AF = mybir.ActivationFunctionType

    nc = bacc.Bacc(
        "TRN2", target_bir_lowering=False, debug=False, num_devices=NCORES
    )
    x_dram = nc.dram_tensor("x_seq", [T, C, P], f16, kind="ExternalInput")
    w_dram = nc.dram_tensor("wmats", [C, 6 * C], f16, kind="ExternalInput")
    b_dram = nc.dram_tensor("biases", [C, 4], f32, kind="ExternalInput")
    o_dram = nc.dram_tensor("out_seq", [T, C, P], f16, kind="ExternalOutput")

    x_ap = x_dram.ap()
    w_ap = w_dram.ap()
    b_ap = b_dram.ap()
    o_ap = o_dram.ap()

    WZX, WZH, WRX, WRH, WHX, WHH = range(6)
    # bias columns: [br, -bz, bh, +bz]
    BR, NBZ, BH, PBZ = range(4)

    def gs(ap_, g):
        return ap_[:, g * PG : (g + 1) * PG]

    with tile.TileContext(nc) as tc:
        with (
            tc.tile_pool(name="consts", bufs=1) as consts,
            tc.tile_pool(name="xin", bufs=4) as xpool,
            tc.tile_pool(name="state", bufs=2) as spool,
            tc.tile_pool(name="work", bufs=2) as wk,
            tc.tile_pool(name="ps", bufs=1, space="PSUM") as ps,
        ):
            bt = consts.tile([C, 4], f32)
            nc.gpsimd.dma_start(bt[:], b_ap[:])
            wt = consts.tile([C, 6 * C], f16)
            nc.sync.dma_start(wt[:], w_ap[:])

            def wslice(i):
                return wt[:, i * C : (i + 1) * C]

            r_ps = ps.tile([C, P], f32, tag="r_ps", bufs=1)
            z_ps = ps.tile([C, P], f32, tag="z_ps", bufs=1)

            def c_tile():
                return ps.tile([C, P], f32, tag="c_ps", bufs=2, name="c_ps")

            def gtile(tag):
                return wk.tile([C, PG], f16, tag=tag, name=tag)

            def htile(g):
                return spool.tile([C, PG], f16, tag=f"h16{g}", name=f"h16{g}")

            # -- preload the ACT table early with a tiny dummy sigmoid on
            #    SBUF data (no PSUM/warmup dependency) --
            wtmp = gtile("r16_0")
            nc.scalar.activation(
                wtmp[:, :4], bt[:, :4], AF.Sigmoid, bias=bt[:, BR : BR + 1]
            )
            c_cur = c_tile()

            def load_x(t):
                xt = xpool.tile([C, P], f16, tag="x", name="x")
                nc.sync.dma_start(xt[:], x_ap[t])
                return xt

            # ---- pipeline stage emitters ----
            def front(g, t, xnext, c_next, h_prev):
                """r/z pre-act close + gates + rh/u/z for (g, t).
                Returns ctx needed by back()."""
                nc.tensor.matmul(gs(r_ps, g), wslice(WRH), h_prev[:],
                                 start=False, stop=True)
                nc.tensor.matmul(gs(z_ps, g), wslice(WZH), h_prev[:],
                                 start=False, stop=True)
                r16 = gtile(f"r16_{g}")
                zb16 = gtile(f"zb16_{g}")
                nc.scalar.activation(r16[:], gs(r_ps, g), AF.Sigmoid,
                                     bias=bt[:, BR : BR + 1])
                nc.scalar.activation(zb16[:], gs(z_ps, g), AF.Sigmoid,
                                     bias=bt[:, NBZ : NBZ + 1], scale=-1.0)
                rh16 = gtile(f"rh16_{g}")
                u16 = gtile(f"u16_{g}")
                z16 = gtile(f"z16_{g}")
                nc.vector.tensor_mul(rh16[:], r16[:], h_prev[:])
                nc.vector.tensor_mul(u16[:], zb16[:], h_prev[:])
                nc.vector.tensor_scalar(z16[:], zb16[:], -1.0, 1.0,
                                        mybir.AluOpType.mult,
                                        mybir.AluOpType.add)
                # next step's r/z x-side opens go later (after the gate
                # reads); they are emitted by opens_rz().
                return {"rh": rh16, "u": u16, "z": z16, "t": t}

            def back_pe(g, ctx, c_ps_t):
                nc.tensor.matmul(gs(c_ps_t, g), wslice(WHH), ctx["rh"][:],
                                 start=False, stop=True)

            def back_rest(g, ctx, c_ps_t):
                t = ctx["t"]
                c16 = gtile(f"c16_{g}")
                nc.scalar.activation(c16[:], gs(c_ps_t, g), AF.Tanh,
                                     bias=bt[:, BH : BH + 1])
                v16 = gtile(f"v16_{g}")
                h_new = htile(g)
                nc.vector.tensor_mul(v16[:], ctx["z"][:], c16[:])
                nc.vector.tensor_add(h_new[:], ctx["u"][:], v16[:])
                nc.sync.dma_start(o_ap[t, :, g * PG : (g + 1) * PG],
                                  h_new[:])
                return h_new

            def open_c(g, xt, c_ps_new):
                nc.tensor.matmul(gs(c_ps_new, g), wslice(WHX), gs(xt, g),
                                 start=True, stop=False)

            def open_rz(g, xt):
                nc.tensor.matmul(gs(r_ps, g), wslice(WRX), gs(xt, g),
                                 start=True, stop=False)
                nc.tensor.matmul(gs(z_ps, g), wslice(WZX), gs(xt, g),
                                 start=True, stop=False)

            # ---------------- t = 0 (h = 0) ----------------
            xs = {0: load_x(0), 1: load_x(1)}
            x0 = xs[0]
            for g in range(G):
                nc.tensor.matmul(gs(z_ps, g), wslice(WZX), gs(x0, g),
                                 start=True, stop=True)
                nc.tensor.matmul(gs(c_cur, g), wslice(WHX), gs(x0, g),
                                 start=True, stop=True)
            xs[2] = load_x(2)
            h_a = None
            h_b = None
            for g in range(G):
                z16 = gtile(f"z16_{g}")
                c16 = gtile(f"c16_{g}")
                nc.scalar.activation(z16[:], gs(z_ps, g), AF.Sigmoid,
                                     bias=bt[:, PBZ : PBZ + 1])
                nc.scalar.activation(c16[:], gs(c_cur, g), AF.Tanh,
                                     bias=bt[:, BH : BH + 1])
                hg = htile(g)
                nc.vector.tensor_mul(hg[:], z16[:], c16[:])
                nc.sync.dma_start(o_ap[0, :, g * PG : (g + 1) * PG], hg[:])
                if g == 0:
                    h_a = hg
                else:
                    h_b = hg

            # opens for step 1 (both groups; banks are free)
            c_cur = c_tile()
            for g in range(G):
                open_c(g, xs[1], c_cur)
                open_rz(g, xs[1])

            # ---------------- pipeline prologue (iter 1) ----------------
            # A full step 1; B front of step 1; opens for 2.
            actx = front(0, 1, xs[2], None, h_a)
            back_pe(0, actx, c_cur)
            back_rest_h = back_rest(0, actx, c_cur)
            h_a = back_rest_h
            bctx = front(1, 1, xs[2], None, h_b)
            c_next = c_tile()
            xs[3] = load_x(3)
            open_c(0, xs[2], c_next)
            open_c(1, xs[2], c_next)
            open_rz(0, xs[2])
            open_rz(1, xs[2])
            c_prev, c_cur = c_cur, c_next

            # ---------------- steady iterations tau = 2..T-1 ----------------
            for t in range(2, T):
                opens = t + 1 < T
                x_n = xs[t + 1] if opens else None
                if t + 2 < T:
                    xs[t + 2] = load_x(t + 2)

                # A front (step t)
                new_actx = front(0, t, x_n, None, h_a)
                # B back (step t-1) -- deps all ready
                back_pe(1, bctx, c_prev)
                # A back (step t)
                back_pe(0, new_actx, c_cur)
                h_b = back_rest(1, bctx, c_prev)
                h_a = back_rest(0, new_actx, c_cur)
                # B front (step t)
                new_bctx = front(1, t, x_n, None, h_b)

                # opens for t+1
                if opens:
                    c_next = c_tile()
                    open_c(0, x_n, c_next)
                    open_rz(0, x_n)
                    open_c(1, x_n, c_next)
                    open_rz(1, x_n)
                    c_prev, c_cur = c_cur, c_next
                else:
                    c_prev = c_cur

                actx, bctx = new_actx, new_bctx

            # ---------------- epilogue: B back of step T-1 ----------------
            back_pe(1, bctx, c_prev)
            back_rest(1, bctx, c_prev)

    nc.compile()
    return nc


def _get_prog():
    global _PROG
    if _PROG is None:
        _PROG = _build()
    return _PROG


def _make_in_maps(video, Wz, bz, Wr, br, Wh, bh):
    w6 = np.concatenate(
        [
            Wz[:, :C].T, Wz[:, C:].T,
            Wr[:, :C].T, Wr[:, C:].T,
            Wh[:, :C].T, Wh[:, C:].T,
        ],
        axis=1,
    ).astype(np.float16)
    b4 = np.stack([br, -bz, bh, bz], axis=1).astype(np.float32)
    in_maps = []
    for core in range(NCORES):
        b_, q = divmod(core, 4)
        xsl = np.ascontiguousarray(
            video[b_, :, :, q * HQ : (q + 1) * HQ, :]
        ).reshape(T, C, P).astype(np.float16)
        in_maps.append({"x_seq": xsl, "wmats": w6, "biases": b4})
    return in_maps


def kernel(video, Wz, bz, Wr, br, Wh, bh):
    _ensure_paths()
    from concourse.bass_utils import run_bass_kernel_spmd

    video = np.asarray(video, dtype=np.float32)
    nc = _get_prog()
    in_maps = _make_in_maps(video, Wz, bz, Wr, br, Wh, bh)
    res = run_bass_kernel_spmd(nc, in_maps, list(range(NCORES)))

    out = np.empty((B, T, C, H, W), np.float32)
    for core in range(NCORES):
        b_, q = divmod(core, 4)
        out[b_, :, :, q * HQ : (q + 1) * HQ, :] = np.asarray(
            res.results[core]["out_seq"]
        ).astype(np.float32).reshape(T, C, HQ, W)
    return out
